# revision 1
# baseline (speedup 1.0000x reference)
"""Trainium2 Bass kernel for BasicPGCBlock:
   per-pixel Gaussian smoothing (5x5, sigma = cubic(perspective)) -> dilated 3x3 conv (256->256) + bias + ReLU.

Sharding: data-parallel over batch, 1 image per NeuronCore (8 cores).

Math: the per-pixel 5x5 kernel w(u,v) = exp(-(u^2+v^2)/(2 s^2)) / Z factors through
t = exp(-1/(2 s^2)):  w(u,v) = t^(u^2+v^2) / Z, and u^2+v^2 in {0,1,2,4,5,8}.
So smoothed = sum_m c_m * S_m with c_m = t^m / Z (host-computed per-pixel planes,
replicated across partitions) and S_m = fixed 0/1 stencil sums of x built from
shifted adds (separable structure). S5 is accumulated on TensorE via identity
matmuls to offload the busiest engine (VectorE). The dilated conv is 9 taps x
(2x2) 128-channel matmul tiles accumulated in PSUM, with bias+ReLU fused into the
ScalarE evacuation.

Layout: channels on partitions (2 tiles of 128 fused as an extra free dim), pixels
on the free dimension, all smoothing in bf16 (DVE 2x mode), conv in bf16 with f32
PSUM accumulation. The last row-slabs are small so only a sliver of conv work
trails the final smoothing op.
"""

import sys

sys.path.insert(0, "/opt/trn_rl_repo")

import numpy as np
import ml_dtypes

BF16 = ml_dtypes.bfloat16

B, C, H, W = 8, 256, 96, 96
HP, WP = H + 4, W + 4          # zero-padded by 2 on each side
SLABS = ((0, 16), (16, 16), (32, 16), (48, 16), (64, 16), (80, 16))
CHUNK = 4                      # conv output rows per matmul (N = 4*96 = 384 <= 512)
OFFS = (-2, 0, 2)              # dilated conv offsets
MS = (0, 1, 2, 4, 5, 8)        # exponents of t present in the 5x5 kernel
PE_S5 = True                   # build S5 = P1[h+-2] + P2[h+-1] sums on TensorE
PE_S28 = False                 # also build S2/S8 row sums on TensorE

_cache = {}


def _build(repeats=1, loop=None, pe_s5=None, pe_s28=None):
    pe_s5 = PE_S5 if pe_s5 is None else pe_s5
    pe_s28 = PE_S28 if pe_s28 is None else pe_s28
    import concourse.mybir as mybir
    from concourse import bacc
    from concourse.tile import TileContext

    dt = mybir.dt
    nc = bacc.Bacc("TRN2", target_bir_lowering=False, debug=False)

    xp = nc.dram_tensor("xp", (128, 2, HP, WP), dt.bfloat16, kind="ExternalInput").ap()
    cpl = nc.dram_tensor("cpl", (128, 6, H, W), dt.bfloat16, kind="ExternalInput").ap()
    wts = nc.dram_tensor("wts", (2, 128, 9 * 2 * 128), dt.bfloat16, kind="ExternalInput").ap()
    bias = nc.dram_tensor("bias", (128, 2), dt.float32, kind="ExternalInput").ap()
    ident = nc.dram_tensor("ident", (128, 128), dt.bfloat16, kind="ExternalInput").ap()
    y = nc.dram_tensor("y", (2, 128, H, W), dt.float32, kind="ExternalOutput").ap()

    with TileContext(nc) as tc:
        with (
            tc.tile_pool(name="const", bufs=1) as constp,
            tc.tile_pool(name="smpool", bufs=1) as smpool,
            tc.tile_pool(name="io", bufs=2) as iop,
            tc.tile_pool(name="tmp", bufs=1) as tmp,
            tc.tile_pool(name="outp", bufs=6) as outp,
            tc.tile_pool(name="psum", bufs=8, space="PSUM") as psp,
        ):
            id_sb = constp.tile([128, 128], dt.bfloat16)
            nc.sync.dma_start(out=id_sb, in_=ident)
            w_sb = constp.tile([128, 2, 9 * 2 * 128], dt.bfloat16)
            b_sb = constp.tile([128, 2], dt.float32)

            def load_consts():
                # emitted after the first slab's input DMAs: conv weights are not
                # needed until well into the first slab, keep them off the
                # startup critical path
                nc.sync.dma_start(out=w_sb[:, 0], in_=wts[0])
                nc.sync.dma_start(out=w_sb[:, 1], in_=wts[1])
                nc.sync.dma_start(out=b_sb, in_=bias)

            sm = smpool.tile([128, 2, HP, WP], dt.bfloat16)
            # zero only the 2-wide pad ring; the interior is fully rewritten
            nc.vector.memset(sm[:, :, 0:2, :], 0.0)
            nc.vector.memset(sm[:, :, HP - 2 : HP, :], 0.0)
            nc.vector.memset(sm[:, :, 2 : HP - 2, 0:2], 0.0)
            nc.vector.memset(sm[:, :, 2 : HP - 2, WP - 2 : WP], 0.0)

            def smooth(r0, nr, flush_fn=None):
                use_pe_s5 = pe_s5 and nr >= 16
                xs = iop.tile([128, 2, nr + 4, WP], dt.bfloat16, name="xs")
                nc.sync.dma_start(out=xs, in_=xp[:, :, r0 : r0 + nr + 4, :])
                cp = iop.tile([128, 6, nr, W], dt.bfloat16, name="cp")
                nc.sync.dma_start(out=cp, in_=cpl[:, :, r0 : r0 + nr, :])

                def cpm(m):
                    i = MS.index(m)
                    return cp[:, i : i + 1].to_broadcast([128, 2, nr, W])

                P0 = xs[:, :, :, 2 : W + 2]
                P1 = tmp.tile([128, 2, nr + 4, W], dt.bfloat16, name="P1", bufs=2)
                nc.vector.tensor_add(P1, xs[:, :, :, 1 : W + 1], xs[:, :, :, 3 : W + 3])
                P2 = tmp.tile([128, 2, nr + 4, W], dt.bfloat16, name="P2", bufs=2)
                nc.vector.tensor_add(P2, xs[:, :, :, 0:W], xs[:, :, :, 4 : W + 4])

                ctr = lambda P: P[:, :, 2 : nr + 2]
                u1 = lambda P: P[:, :, 1 : nr + 1]
                d1 = lambda P: P[:, :, 3 : nr + 3]
                u2 = lambda P: P[:, :, 0:nr]
                d2 = lambda P: P[:, :, 4 : nr + 4]

                if use_pe_s5:
                    # S5 = (P1[h-2]+P1[h+2]) + (P2[h-1]+P2[h+1]) accumulated on
                    # TensorE via identity matmuls, evacuated to bf16 by ScalarE.
                    S5 = tmp.tile([128, 2, nr, W], dt.bfloat16, name="S5", bufs=2)
                    for ct in range(2):
                        for rk in range(nr // CHUNK):
                            rs = CHUNK * rk
                            pc5 = psp.tile(
                                [128, CHUNK, W], dt.float32, name="pc5", bufs=2
                            )
                            for j, Pv in enumerate((u2(P1), d2(P1), u1(P2), d1(P2))):
                                nc.tensor.matmul(
                                    pc5,
                                    id_sb,
                                    Pv[:, ct, rs : rs + CHUNK, :],
                                    start=(j == 0),
                                    stop=(j == 3),
                                )
                            nc.scalar.activation(
                                S5[:, ct, rs : rs + CHUNK, :],
                                pc5,
                                mybir.ActivationFunctionType.Copy,
                            )


                if flush_fn is not None:
                    # last slab: build all stencil sums as tiles, then apply in
                    # two row-halves with a conv flush in between so most of the
                    # trailing conv work starts before smoothing finishes.
                    S1 = tmp.tile([128, 2, nr, W], dt.bfloat16, name="S1")
                    nc.vector.tensor_add(S1, u1(P0), d1(P0))
                    nc.vector.tensor_add(S1, S1, ctr(P1))
                    S2 = tmp.tile([128, 2, nr, W], dt.bfloat16, name="S2")
                    nc.vector.tensor_add(S2, u1(P1), d1(P1))
                    S4 = tmp.tile([128, 2, nr, W], dt.bfloat16, name="S4")
                    nc.vector.tensor_add(S4, u2(P0), d2(P0))
                    nc.vector.tensor_add(S4, S4, ctr(P2))
                    if not use_pe_s5:
                        S5 = tmp.tile([128, 2, nr, W], dt.bfloat16, name="S5x")
                        nc.vector.tensor_add(S5, u2(P1), d2(P1))
                        Qb = tmp.tile([128, 2, nr, W], dt.bfloat16, name="Qb")
                        nc.vector.tensor_add(Qb, u1(P2), d1(P2))
                        nc.vector.tensor_add(S5, S5, Qb)
                    S8 = tmp.tile([128, 2, nr, W], dt.bfloat16, name="S8")
                    nc.vector.tensor_add(S8, u2(P2), d2(P2))
                    hn = nr // 2
                    for h0 in (0, hn):
                        if h0:
                            flush_fn(r0 + h0)

                        def cpmh(m):
                            i = MS.index(m)
                            return cp[:, i : i + 1, h0 : h0 + hn, :].to_broadcast(
                                [128, 2, hn, W]
                            )

                        acc = tmp.tile([128, 2, hn, W], dt.bfloat16, name="acc")
                        nc.vector.tensor_mul(
                            acc, ctr(P0)[:, :, h0 : h0 + hn], cpmh(0)
                        )
                        smo = sm[:, :, 2 + r0 + h0 : 2 + r0 + h0 + hn, 2 : W + 2]
                        for Sx, m in ((S1, 1), (S2, 2), (S4, 4), (S5, 5), (S8, 8)):
                            t = tmp.tile([128, 2, hn, W], dt.bfloat16, name="t")
                            nc.vector.tensor_mul(t, Sx[:, :, h0 : h0 + hn], cpmh(m))
                            nc.vector.tensor_add(smo if m == 8 else acc, acc, t)
                    return

                acc = tmp.tile([128, 2, nr, W], dt.bfloat16, name="acc")
                nc.vector.tensor_mul(acc, ctr(P0), cpm(0))

                sm_out = sm[:, :, 2 + r0 : 2 + r0 + nr, 2 : W + 2]

                def term(S, m, last=False):
                    t = tmp.tile([128, 2, nr, W], dt.bfloat16, name="t")
                    nc.vector.tensor_mul(t, S, cpm(m))
                    nc.vector.tensor_add(sm_out if last else acc, acc, t)

                # m=1: S1 = (P0[h-1]+P0[h+1]) + P1[h]
                Qa = tmp.tile([128, 2, nr, W], dt.bfloat16, name="Qa")
                nc.vector.tensor_add(Qa, u1(P0), d1(P0))
                S = tmp.tile([128, 2, nr, W], dt.bfloat16, name="S")
                nc.vector.tensor_add(S, Qa, ctr(P1))
                term(S, 1)
                # m=2: S2 = P1[h-1]+P1[h+1]
                S = tmp.tile([128, 2, nr, W], dt.bfloat16, name="S")
                nc.vector.tensor_add(S, u1(P1), d1(P1))
                term(S, 2)
                # m=4: S4 = (P0[h-2]+P0[h+2]) + P2[h]
                Qa = tmp.tile([128, 2, nr, W], dt.bfloat16, name="Qa")
                nc.vector.tensor_add(Qa, u2(P0), d2(P0))
                S = tmp.tile([128, 2, nr, W], dt.bfloat16, name="S")
                nc.vector.tensor_add(S, Qa, ctr(P2))
                term(S, 4)
                # m=5
                if use_pe_s5:
                    term(S5, 5)
                else:
                    Qa = tmp.tile([128, 2, nr, W], dt.bfloat16, name="Qa")
                    nc.vector.tensor_add(Qa, u2(P1), d2(P1))
                    Qb = tmp.tile([128, 2, nr, W], dt.bfloat16, name="Qb")
                    nc.vector.tensor_add(Qb, u1(P2), d1(P2))
                    S = tmp.tile([128, 2, nr, W], dt.bfloat16, name="S")
                    nc.vector.tensor_add(S, Qa, Qb)
                    term(S, 5)
                # m=8: S8 = P2[h-2]+P2[h+2]
                S = tmp.tile([128, 2, nr, W], dt.bfloat16, name="S")
                nc.vector.tensor_add(S, u2(P2), d2(P2))
                term(S, 8, last=True)

            def conv_group(rrs):
                # rrs: output-row starts whose sm dependencies are met; one
                # LDWEIGHTS serves len(rrs) matmuls.
                for oi in range(2):
                    pcs = [
                        psp.tile([128, CHUNK, W], dt.float32, name="pc", bufs=6)
                        for _ in rrs
                    ]
                    for idx in range(18):
                        ki, q = idx // 9, idx % 9
                        dh, dw = OFFS[q // 3], OFFS[q % 3]
                        lhsT = w_sb[:, ki, (q * 2 + oi) * 128 : (q * 2 + oi + 1) * 128]
                        for j, rr in enumerate(rrs):
                            rhs = sm[
                                :, ki, 2 + rr + dh : 2 + rr + CHUNK + dh, 2 + dw : 2 + dw + W
                            ]
                            nc.tensor.matmul(
                                pcs[j], lhsT, rhs, start=(idx == 0), stop=(idx == 17)
                            )
                    for j, rr in enumerate(rrs):
                        ob = outp.tile([128, CHUNK, W], dt.float32, name="ob")
                        nc.scalar.activation(
                            ob,
                            pcs[j],
                            mybir.ActivationFunctionType.Relu,
                            bias=b_sb[:, oi : oi + 1],
                            scale=1.0,
                        )
                        nc.sync.dma_start(out=y[oi, :, rr : rr + CHUNK, :], in_=ob)

            def body():
                # conv rows rr..rr+3 read smp rows rr..rr+7 (interior rr-2..rr+5):
                # emit each chunk as soon as smoothing covers row rr+5.
                pending = list(range(0, H, CHUNK))
                def flush(upto):
                    ready = [rr for rr in pending if rr + 6 <= upto or upto >= H]
                    for rr in ready:
                        pending.remove(rr)
                    if ready:
                        conv_group(ready)

                for si, (r0, nr) in enumerate(SLABS):
                    # last-slab split apply measured no better on HW and the
                    # model agrees post-double-buffering: the tail is PE-queue
                    # bound, not dependency bound. Keep single-pass emission.
                    smooth(r0, nr, flush_fn=None)
                    if si == 0:
                        load_consts()
                    flush(r0 + nr)
                assert not pending

            if loop is not None:
                with tc.For_i(0, loop, 1):
                    body()
            else:
                for _ in range(repeats):
                    body()

    nc.compile()
    return nc


def _prep(inputs):
    x = np.asarray(inputs["x"], np.float32)
    pm = np.asarray(inputs["perspective_map"], np.float32)
    co = np.asarray(inputs["sigma_coeffs"], np.float32)
    Wc = np.asarray(inputs["conv_w"], np.float32)
    bb = np.asarray(inputs["conv_b"], np.float32)

    # per-pixel coefficient planes (host): c_m = t^m / Z, replicated over partitions
    p = pm[:, 0]  # [B,H,W]
    sigma = co[0] * p**3 + co[1] * p**2 + co[2] * p + co[3]
    sigma = np.maximum(sigma, 0.5)
    t = np.exp(-1.0 / (2.0 * sigma * sigma))
    Z = 1 + 4 * t + 4 * t**2 + 4 * t**4 + 8 * t**5 + 4 * t**8
    cm = np.stack([(t**m) / Z for m in MS], axis=1).astype(BF16)  # [B,6,H,W]
    cpl = np.ascontiguousarray(np.broadcast_to(cm[:, None], (B, 128, 6, H, W)))

    # zero-padded bf16 input: [B, 128(part), 2(ct), HP, WP]
    xpad = np.zeros((B, 128, 2, HP, WP), BF16)
    xpad[:, :, :, 2 : H + 2, 2 : W + 2] = (
        x.astype(BF16).reshape(B, 2, 128, H, W).transpose(0, 2, 1, 3, 4)
    )

    # conv weights: lhsT layout [ki, 128(i), q, oi, 128(o)]
    Wt = Wc.transpose(1, 0, 2, 3).astype(BF16)  # [I, O, kh, kw]
    wts = np.empty((2, 128, 9, 2, 128), BF16)
    for ki in range(2):
        for q in range(9):
            kh, kw = q // 3, q % 3
            for oi in range(2):
                wts[ki, :, q, oi, :] = Wt[
                    ki * 128 : (ki + 1) * 128, oi * 128 : (oi + 1) * 128, kh, kw
                ]
    wts = wts.reshape(2, 128, 9 * 2 * 128)
    bias_h = np.ascontiguousarray(bb.reshape(2, 128).T.astype(np.float32))  # [128, 2]
    ident = np.eye(128, dtype=BF16)

    return [
        {"xp": xpad[b], "cpl": cpl[b], "wts": wts, "bias": bias_h, "ident": ident}
        for b in range(B)
    ]


def _get_nc(repeats=1, loop=None, pe_s5=None, pe_s28=None):
    key = ("nc", repeats, loop, pe_s5, pe_s28)
    if key not in _cache:
        _cache[key] = _build(repeats, loop, pe_s5, pe_s28)
    return _cache[key]


def run(inputs, trace=False, **kw):
    from concourse.bass_utils import run_bass_kernel_spmd

    nc = _get_nc()
    in_maps = _prep(inputs)
    res = run_bass_kernel_spmd(nc, in_maps, core_ids=list(range(B)), trace=trace, **kw)
    out = np.stack([r["y"].reshape(C, H, W) for r in res.results]).astype(np.float32)
    return out, res


def kernel(**inputs):
    out, _ = run(inputs)
    return out



# revision 24
# speedup vs baseline: 243.4713x; 243.4713x over previous
"""Trainium2 Bass kernel for BasicPGCBlock:
   per-pixel Gaussian smoothing (5x5, sigma = cubic(perspective)) -> dilated 3x3 conv (256->256) + bias + ReLU.

Sharding: data-parallel over batch, 1 image per NeuronCore (8 cores).

Math: the per-pixel 5x5 kernel w(u,v) = exp(-(u^2+v^2)/(2 s^2)) / Z factors through
t = exp(-1/(2 s^2)):  w(u,v) = t^(u^2+v^2) / Z, and u^2+v^2 in {0,1,2,4,5,8}.
So smoothed = sum_m c_m * S_m with c_m = t^m / Z (host-computed per-pixel planes,
replicated across partitions) and S_m = fixed 0/1 stencil sums of x built from
shifted adds (separable structure).

Engine split (the kernel is PE-sequencer- and DVE-bound; cost model says each
matmul costs ~223ns of PE SEQ regardless of engine time):
 - PE: conv only, 5-row output groups (N=480 moving) -> 720 matmuls.
 - DVE: P1/P2 + S1/S2/S4/S8 stencils + the c_m MAC chain (bf16 2x mode).
 - Pool (gpsimd): the 3-add S5 stencil (S5 is the widest sum); MAC term order
   puts m=5 last so Pool has a full slab period of slack.
 - Act: PSUM evacuation with fused bias+ReLU, bf16 output (halves y DMA).

Layout: channels on partitions (2 tiles of 128 fused as an extra free dim),
pixels on the free dimension, all smoothing in bf16, conv bf16 with f32 PSUM.
The last slab applies the MAC chain in a 6+2 row split so the conv tail after
the final smoothing op is one 4-row group.
"""

import sys

sys.path.insert(0, "/opt/trn_rl_repo")

import numpy as np
import ml_dtypes

BF16 = ml_dtypes.bfloat16

B, C, H, W = 8, 256, 96, 96
HP, WP = H + 4, W + 4          # zero-padded by 2 on each side
# (row0, nrows, parts): each part is (h0, hn, final-add slice boundaries) —
# the MAC chain is applied per part (so late parts release conv early), and
# within a part the final add is emitted per slice. The 8-row first slab gets
# the first conv group started ~20us earlier; the last slab is split 6+2 so
# the very last conv work is a single 4-row group.
SLABS = (
    (0, 8, ((0, 8, ((0, 8),)),)),
    (8, 16, ((0, 16, ((0, 6), (6, 11), (11, 16))),)),
    (24, 16, ((0, 16, ((0, 6), (6, 11), (11, 16))),)),
    (40, 16, ((0, 16, ((0, 6), (6, 11), (11, 16))),)),
    (56, 16, ((0, 16, ((0, 6), (6, 11), (11, 16))),)),
    (72, 16, ((0, 16, ((0, 6), (6, 11), (11, 16))),)),
    (88, 8, ((0, 6, ((0, 4), (4, 6))), (6, 2, ((6, 8),)))),
)
OFFS = (-2, 0, 2)              # dilated conv offsets
MS = (0, 1, 2, 4, 5, 8)        # exponents of t present in the 5x5 kernel
# conv output row-groups (start, nrows): 18x5 + (90,2) + (92,4); the final
# 4-row group is the only conv work gated on the last 2 smoothed rows.
CGROUPS = tuple((i * 5, 5) for i in range(18)) + ((90, 2), (92, 4))

_cache = {}


def _build(repeats=1, loop=None):
    import concourse.mybir as mybir
    from concourse import bacc
    from concourse.tile import TileContext

    dt = mybir.dt
    nc = bacc.Bacc("TRN2", target_bir_lowering=False, debug=False)

    xp = nc.dram_tensor("xp", (128, 2, HP, WP), dt.bfloat16, kind="ExternalInput").ap()
    cpl = nc.dram_tensor("cpl", (128, 6, H, W), dt.bfloat16, kind="ExternalInput").ap()
    wts = nc.dram_tensor("wts", (2, 128, 9 * 2 * 128), dt.bfloat16, kind="ExternalInput").ap()
    bias = nc.dram_tensor("bias", (128, 2), dt.float32, kind="ExternalInput").ap()
    y = nc.dram_tensor("y", (2, 128, H, W), dt.bfloat16, kind="ExternalOutput").ap()

    with TileContext(nc) as tc:
        with (
            tc.tile_pool(name="const", bufs=1) as constp,
            tc.tile_pool(name="smpool", bufs=1) as smpool,
            tc.tile_pool(name="io", bufs=2) as iop,
            tc.tile_pool(name="tmp", bufs=1) as tmp,
            tc.tile_pool(name="outp", bufs=1) as outp,
            tc.tile_pool(name="psum", bufs=1, space="PSUM") as psp,
        ):
            w_sb = constp.tile([128, 2, 9 * 2 * 128], dt.bfloat16)
            b_sb = constp.tile([128, 2], dt.float32)

            def load_consts():
                # Activation-engine DMA queue: runs in parallel with the SP
                # queue that carries the (much larger) xs/cp input stream, so
                # the first conv group is never gated on the weights landing.
                nc.scalar.dma_start(out=w_sb[:, 0], in_=wts[0])
                nc.scalar.dma_start(out=w_sb[:, 1], in_=wts[1])
                nc.scalar.dma_start(out=b_sb, in_=bias)

            sm = smpool.tile([128, 2, HP, WP], dt.bfloat16)
            # zero only the 2-wide pad ring; the interior is fully rewritten
            nc.vector.memset(sm[:, :, 0:2, :], 0.0)
            nc.vector.memset(sm[:, :, HP - 2 : HP, :], 0.0)
            nc.vector.memset(sm[:, :, 2 : HP - 2, 0:2], 0.0)
            nc.vector.memset(sm[:, :, 2 : HP - 2, WP - 2 : WP], 0.0)

            def smooth(r0, nr, flush_fn=None, parts=None):
                xs = iop.tile([128, 2, nr + 4, WP], dt.bfloat16, name="xs")
                nc.sync.dma_start(out=xs, in_=xp[:, :, r0 : r0 + nr + 4, :])
                cp = iop.tile([128, 6, nr, W], dt.bfloat16, name="cp")
                nc.sync.dma_start(out=cp, in_=cpl[:, :, r0 : r0 + nr, :])

                P0 = xs[:, :, :, 2 : W + 2]
                P1 = tmp.tile([128, 2, nr + 4, W], dt.bfloat16, name="P1", bufs=2)
                nc.vector.tensor_add(P1, xs[:, :, :, 1 : W + 1], xs[:, :, :, 3 : W + 3])
                P2 = tmp.tile([128, 2, nr + 4, W], dt.bfloat16, name="P2", bufs=2)
                nc.vector.tensor_add(P2, xs[:, :, :, 0:W], xs[:, :, :, 4 : W + 4])

                ctr = lambda P: P[:, :, 2 : nr + 2]
                u1 = lambda P: P[:, :, 1 : nr + 1]
                d1 = lambda P: P[:, :, 3 : nr + 3]
                u2 = lambda P: P[:, :, 0:nr]
                d2 = lambda P: P[:, :, 4 : nr + 4]

                # Pool engine: S8 = P2[h-2]+P2[h+2] (needed mid-MAC, so first)
                # then S5 = (P1[h-2]+P1[h+2]) + (P2[h-1]+P2[h+1]) (needed by
                # the last MAC term, so Pool has ~a slab period of slack).
                S8 = tmp.tile([128, 2, nr, W], dt.bfloat16, name="S8", bufs=2)
                nc.gpsimd.tensor_add(S8, u2(P2), d2(P2))
                S5 = tmp.tile([128, 2, nr, W], dt.bfloat16, name="S5", bufs=2)
                Qp = tmp.tile([128, 2, nr, W], dt.bfloat16, name="Qp", bufs=2)
                nc.gpsimd.tensor_add(S5, u2(P1), d2(P1))
                nc.gpsimd.tensor_add(Qp, u1(P2), d1(P2))
                nc.gpsimd.tensor_add(S5, S5, Qp)

                S1 = tmp.tile([128, 2, nr, W], dt.bfloat16, name="S1")
                nc.vector.tensor_add(S1, u1(P0), d1(P0))
                nc.vector.tensor_add(S1, S1, ctr(P1))
                S2 = tmp.tile([128, 2, nr, W], dt.bfloat16, name="S2")
                nc.vector.tensor_add(S2, u1(P1), d1(P1))
                S4 = tmp.tile([128, 2, nr, W], dt.bfloat16, name="S4")
                nc.vector.tensor_add(S4, u2(P0), d2(P0))
                nc.vector.tensor_add(S4, S4, ctr(P2))

                acc = tmp.tile([128, 2, nr, W], dt.bfloat16, name="acc")

                for h0, hn, slices in parts:
                    hs = slice(h0, h0 + hn)

                    def cpm(m):
                        i = MS.index(m)
                        return cp[:, i : i + 1, hs].to_broadcast([128, 2, hn, W])

                    av = acc[:, :, hs]
                    nc.vector.tensor_mul(av, ctr(P0)[:, :, hs], cpm(0))
                    # m=8 mid-chain (Pool's S8 lands early), m=5 last (Pool
                    # has ~a slab of slack)
                    tv = None
                    for Sx, m in ((S1, 1), (S2, 2), (S8, 8), (S4, 4), (S5, 5)):
                        tv = tmp.tile([128, 2, nr, W], dt.bfloat16, name="t", bufs=2)
                        nc.vector.tensor_mul(tv[:, :, hs], Sx[:, :, hs], cpm(m))
                        if m != 5:
                            nc.vector.tensor_add(av, av, tv[:, :, hs])
                    for a, b in slices:
                        nc.vector.tensor_add(
                            sm[:, :, 2 + r0 + a : 2 + r0 + b, 2 : W + 2],
                            acc[:, :, a:b],
                            tv[:, :, a:b],
                        )
                        if flush_fn is not None:
                            flush_fn(r0 + b)

            def conv_group(groups):
                # groups: (rr, gn) output-row groups whose sm rows are ready
                for oi in range(2):
                    for rr, gn in groups:
                        nb = 6 if gn == 5 else 1
                        pc = psp.tile([128, gn, W], dt.float32, name=f"pc{gn}", bufs=nb)
                        for idx in range(18):
                            ki, q = idx // 9, idx % 9
                            dh, dw = OFFS[q // 3], OFFS[q % 3]
                            lhsT = w_sb[:, ki, (q * 2 + oi) * 128 : (q * 2 + oi + 1) * 128]
                            rhs = sm[
                                :, ki, 2 + rr + dh : 2 + rr + gn + dh, 2 + dw : 2 + dw + W
                            ]
                            nc.tensor.matmul(
                                pc, lhsT, rhs, start=(idx == 0), stop=(idx == 17)
                            )
                        ob = outp.tile([128, gn, W], dt.bfloat16, name=f"ob{gn}", bufs=(4 if gn == 5 else 2))
                        nc.scalar.activation(
                            ob,
                            pc,
                            mybir.ActivationFunctionType.Relu,
                            bias=b_sb[:, oi : oi + 1],
                            scale=1.0,
                        )
                        nc.sync.dma_start(out=y[oi, :, rr : rr + gn, :], in_=ob)

            def body():
                pending = list(CGROUPS)

                def flush(upto):
                    # group (rr, gn) reads sm rows rr-2 .. rr+gn+1 (dilated
                    # taps); rows 0..upto-1 have been written
                    ready = [g for g in pending if min(g[0] + g[1] + 2, H) <= upto]
                    for g in ready:
                        pending.remove(g)
                    if ready:
                        conv_group(ready)

                load_consts()
                for r0, nr, parts in SLABS:
                    smooth(r0, nr, flush_fn=flush, parts=parts)
                assert not pending

            if loop is not None:
                with tc.For_i(0, loop, 1):
                    body()
            else:
                for _ in range(repeats):
                    body()

    nc.compile()
    return nc


def _prep(inputs):
    x = np.asarray(inputs["x"], np.float32)
    pm = np.asarray(inputs["perspective_map"], np.float32)
    co = np.asarray(inputs["sigma_coeffs"], np.float32)
    Wc = np.asarray(inputs["conv_w"], np.float32)
    bb = np.asarray(inputs["conv_b"], np.float32)

    # per-pixel coefficient planes (host): c_m = t^m / Z, replicated over partitions
    p = pm[:, 0]  # [B,H,W]
    sigma = co[0] * p**3 + co[1] * p**2 + co[2] * p + co[3]
    sigma = np.maximum(sigma, 0.5)
    t = np.exp(-1.0 / (2.0 * sigma * sigma))
    Z = 1 + 4 * t + 4 * t**2 + 4 * t**4 + 8 * t**5 + 4 * t**8
    cm = np.stack([(t**m) / Z for m in MS], axis=1).astype(BF16)  # [B,6,H,W]
    cpl = np.ascontiguousarray(np.broadcast_to(cm[:, None], (B, 128, 6, H, W)))

    # zero-padded bf16 input: [B, 128(part), 2(ct), HP, WP]
    xpad = np.zeros((B, 128, 2, HP, WP), BF16)
    xpad[:, :, :, 2 : H + 2, 2 : W + 2] = (
        x.astype(BF16).reshape(B, 2, 128, H, W).transpose(0, 2, 1, 3, 4)
    )

    # conv weights: lhsT layout [ki, 128(i), q, oi, 128(o)]
    Wt = Wc.transpose(1, 0, 2, 3).astype(BF16)  # [I, O, kh, kw]
    wts = np.empty((2, 128, 9, 2, 128), BF16)
    for ki in range(2):
        for q in range(9):
            kh, kw = q // 3, q % 3
            for oi in range(2):
                wts[ki, :, q, oi, :] = Wt[
                    ki * 128 : (ki + 1) * 128, oi * 128 : (oi + 1) * 128, kh, kw
                ]
    wts = wts.reshape(2, 128, 9 * 2 * 128)
    bias_h = np.ascontiguousarray(bb.reshape(2, 128).T.astype(np.float32))  # [128, 2]

    return [
        {"xp": xpad[b], "cpl": cpl[b], "wts": wts, "bias": bias_h}
        for b in range(B)
    ]


def _get_nc(repeats=1, loop=None):
    key = ("nc", repeats, loop)
    if key not in _cache:
        _cache[key] = _build(repeats, loop)
    return _cache[key]


def run(inputs, trace=False, **kw):
    from concourse.bass_utils import run_bass_kernel_spmd

    nc = _get_nc()
    in_maps = _prep(inputs)
    res = run_bass_kernel_spmd(nc, in_maps, core_ids=list(range(B)), trace=trace, **kw)
    out = np.stack([r["y"].reshape(C, H, W) for r in res.results]).astype(np.float32)
    return out, res


def kernel(**inputs):
    out, _ = run(inputs)
    return out


# revision 33
# speedup vs baseline: 423.3042x; 1.7386x over previous
"""Trainium2 Bass kernel for BasicPGCBlock:
   per-pixel Gaussian smoothing (5x5, sigma = cubic(perspective)) -> dilated 3x3 conv (256->256) + bias + ReLU.

Sharding: data-parallel over batch, 1 image per NeuronCore (8 cores).

Math: the per-pixel 5x5 kernel w(u,v) = exp(-(u^2+v^2)/(2 s^2)) / Z factors through
t = exp(-1/(2 s^2)):  w(u,v) = t^(u^2+v^2) / Z, and u^2+v^2 in {0,1,2,4,5,8}.
So smoothed = sum_m c_m * S_m with c_m = t^m / Z (host-computed per-pixel planes,
replicated across partitions) and S_m = fixed 0/1 stencil sums of x built from
shifted adds (separable structure).

Engine split (the kernel is PE-sequencer- and DVE-bound; cost model says each
matmul costs ~223ns of PE SEQ regardless of engine time):
 - PE: conv only, 5-row output groups (N=480 moving) -> 720 matmuls.
 - DVE: P1/P2 + S1/S2/S4/S8 stencils + the c_m MAC chain (bf16 2x mode).
 - Pool (gpsimd): the 3-add S5 stencil (S5 is the widest sum); MAC term order
   puts m=5 last so Pool has a full slab period of slack.
 - Act: PSUM evacuation with fused bias+ReLU, bf16 output (halves y DMA).

Layout: channels on partitions (2 tiles of 128 fused as an extra free dim),
pixels on the free dimension, all smoothing in bf16, conv bf16 with f32 PSUM.
The last slab applies the MAC chain in a 6+2 row split so the conv tail after
the final smoothing op is one 4-row group.
"""

import sys

sys.path.insert(0, "/opt/trn_rl_repo")

import numpy as np
import ml_dtypes

BF16 = ml_dtypes.bfloat16

B, C, H, W = 8, 256, 96, 96
HP, WP = H + 4, W + 4          # zero-padded by 2 on each side
# (row0, nrows, parts): each part is (h0, hn, final-add slice boundaries) —
# the MAC chain is applied per part (so late parts release conv early), and
# within a part the final add is emitted per slice. The 8-row first slab gets
# the first conv group started ~20us earlier; the last slab is split 6+2 so
# the very last conv work is a single 4-row group.
SLABS = (
    (0, 8, ((0, 8, ((0, 8),)),)),
    (8, 16, ((0, 16, ((0, 6), (6, 11), (11, 16))),)),
    (24, 16, ((0, 16, ((0, 6), (6, 11), (11, 16))),)),
    (40, 16, ((0, 16, ((0, 6), (6, 11), (11, 16))),)),
    (56, 16, ((0, 16, ((0, 6), (6, 11), (11, 16))),)),
    (72, 16, ((0, 16, ((0, 6), (6, 11), (11, 16))),)),
    (88, 8, ((0, 6, ((0, 4), (4, 6))), (6, 2, ((6, 8),)))),
)
OFFS = (-2, 0, 2)              # dilated conv offsets
MS = (0, 1, 2, 4, 5, 8)        # exponents of t present in the 5x5 kernel
# conv output row-groups (start, nrows): 18x5 + (90,2) + (92,4); the final
# 4-row group is the only conv work gated on the last 2 smoothed rows.
CGROUPS = tuple((i * 5, 5) for i in range(18)) + ((90, 2), (92, 4))

_cache = {}


def _build(repeats=1, loop=None, s5="pe", worder=True, chunk=5, wq="act", yf32=False, slabs="s7", slices="fine"):
    import concourse.mybir as mybir
    from concourse import bacc
    from concourse.tile import TileContext

    if chunk == 5:
        cgroups = CGROUPS
    else:
        cgroups = tuple((i * 4, 4) for i in range(24))

    def mid_parts(nr):
        if slices == "fine":
            bnds = ((0, 6), (6, 11), (11, 16)) if nr == 16 else ((0, 8),)
            return ((0, nr, bnds),)
        return ((0, nr, ((0, nr),)),)

    if slabs == "s7":
        slab_list = [(0, 8, mid_parts(8))] + [
            (r, 16, mid_parts(16)) for r in (8, 24, 40, 56, 72)
        ] + [(88, 8, ((0, 6, (((0, 4), (4, 6)) if slices == "fine" else ((0, 6),))),
                      (6, 2, ((6, 8),))))]
    else:
        slab_list = [(r, 16, mid_parts(16)) for r in (0, 16, 32, 48, 64)] + [
            (80, 16, ((0, 14, (((0, 6), (6, 11), (11, 14)) if slices == "fine" else ((0, 14),))),
                      (14, 2, ((14, 16),))))
        ]
    dt = mybir.dt
    nc = bacc.Bacc("TRN2", target_bir_lowering=False, debug=False)

    xp = nc.dram_tensor("xp", (128, 2, HP, WP), dt.bfloat16, kind="ExternalInput").ap()
    cpl = nc.dram_tensor("cpl", (128, 6, H, W), dt.bfloat16, kind="ExternalInput").ap()
    wts = nc.dram_tensor("wts", (2, 128, 9 * 2 * 128), dt.bfloat16, kind="ExternalInput").ap()
    bias = nc.dram_tensor("bias", (128, 2), dt.float32, kind="ExternalInput").ap()
    ident = nc.dram_tensor("ident", (128, 128), dt.bfloat16, kind="ExternalInput").ap()
    ydt = dt.float32 if yf32 else dt.bfloat16
    y = nc.dram_tensor("y", (2, 128, H, W), ydt, kind="ExternalOutput").ap()

    with TileContext(nc) as tc:
        with (
            tc.tile_pool(name="const", bufs=1) as constp,
            tc.tile_pool(name="smpool", bufs=1) as smpool,
            tc.tile_pool(name="io", bufs=2) as iop,
            tc.tile_pool(name="tmp", bufs=1) as tmp,
            tc.tile_pool(name="outp", bufs=1) as outp,
            tc.tile_pool(name="psum", bufs=1, space="PSUM") as psp,
        ):
            w_sb = constp.tile([128, 2, 9 * 2 * 128], dt.bfloat16)
            b_sb = constp.tile([128, 2], dt.float32)
            id_sb = constp.tile([128, 128], dt.bfloat16)
            if s5 == "pe":
                nc.sync.dma_start(out=id_sb, in_=ident)

            def load_consts():
                # Activation-engine DMA queue: runs in parallel with the SP
                # queue that carries the (much larger) xs/cp input stream, so
                # the first conv group is never gated on the weights landing.
                dq = nc.scalar if wq == "act" else nc.sync
                dq.dma_start(out=w_sb[:, 0], in_=wts[0])
                dq.dma_start(out=w_sb[:, 1], in_=wts[1])
                dq.dma_start(out=b_sb, in_=bias)

            sm = smpool.tile([128, 2, HP, WP], dt.bfloat16)
            # zero only the 2-wide pad ring; the interior is fully rewritten
            nc.vector.memset(sm[:, :, 0:2, :], 0.0)
            nc.vector.memset(sm[:, :, HP - 2 : HP, :], 0.0)
            nc.vector.memset(sm[:, :, 2 : HP - 2, 0:2], 0.0)
            nc.vector.memset(sm[:, :, 2 : HP - 2, WP - 2 : WP], 0.0)

            def smooth(r0, nr, flush_fn=None, parts=None):
                xs = iop.tile([128, 2, nr + 4, WP], dt.bfloat16, name="xs")
                nc.sync.dma_start(out=xs, in_=xp[:, :, r0 : r0 + nr + 4, :])
                cp = iop.tile([128, 6, nr, W], dt.bfloat16, name="cp")
                nc.sync.dma_start(out=cp, in_=cpl[:, :, r0 : r0 + nr, :])

                P0 = xs[:, :, :, 2 : W + 2]
                P1 = tmp.tile([128, 2, nr + 4, W], dt.bfloat16, name="P1", bufs=2)
                nc.vector.tensor_add(P1, xs[:, :, :, 1 : W + 1], xs[:, :, :, 3 : W + 3])
                P2 = tmp.tile([128, 2, nr + 4, W], dt.bfloat16, name="P2", bufs=2)
                nc.vector.tensor_add(P2, xs[:, :, :, 0:W], xs[:, :, :, 4 : W + 4])

                ctr = lambda P: P[:, :, 2 : nr + 2]
                u1 = lambda P: P[:, :, 1 : nr + 1]
                d1 = lambda P: P[:, :, 3 : nr + 3]
                u2 = lambda P: P[:, :, 0:nr]
                d2 = lambda P: P[:, :, 4 : nr + 4]

                # S5 = (P1[h-2]+P1[h+2]) + (P2[h-1]+P2[h+1]): on PE via
                # identity-matmul PSUM accumulation (offloads the busiest
                # engine, DVE), with gpsimd/DVE fallbacks for A/B testing.
                S5 = tmp.tile([128, 2, nr, W], dt.bfloat16, name="S5", bufs=2)
                S8 = tmp.tile([128, 2, nr, W], dt.bfloat16, name="S8", bufs=2)
                if s5 == "pe":
                    nc.vector.tensor_add(S8, u2(P2), d2(P2))
                    for ct in range(2):
                        for rs in range(0, nr, 4):
                            pc5 = psp.tile([128, 4, W], dt.float32, name="ps5", bufs=2)
                            for j, Pv in enumerate((u2(P1), d2(P1), u1(P2), d1(P2))):
                                nc.tensor.matmul(
                                    pc5, id_sb, Pv[:, ct, rs : rs + 4, :],
                                    start=(j == 0), stop=(j == 3),
                                )
                            nc.scalar.activation(
                                S5[:, ct, rs : rs + 4, :], pc5,
                                mybir.ActivationFunctionType.Copy,
                            )
                else:
                    eng = nc.gpsimd if s5 == "pool" else nc.vector
                    eng.tensor_add(S8, u2(P2), d2(P2))
                    Qp = tmp.tile([128, 2, nr, W], dt.bfloat16, name="Qp", bufs=2)
                    eng.tensor_add(S5, u2(P1), d2(P1))
                    eng.tensor_add(Qp, u1(P2), d1(P2))
                    eng.tensor_add(S5, S5, Qp)

                S1 = tmp.tile([128, 2, nr, W], dt.bfloat16, name="S1")
                nc.vector.tensor_add(S1, u1(P0), d1(P0))
                nc.vector.tensor_add(S1, S1, ctr(P1))
                S2 = tmp.tile([128, 2, nr, W], dt.bfloat16, name="S2")
                nc.vector.tensor_add(S2, u1(P1), d1(P1))
                S4 = tmp.tile([128, 2, nr, W], dt.bfloat16, name="S4")
                nc.vector.tensor_add(S4, u2(P0), d2(P0))
                nc.vector.tensor_add(S4, S4, ctr(P2))

                acc = tmp.tile([128, 2, nr, W], dt.bfloat16, name="acc")

                for h0, hn, slices in parts:
                    hs = slice(h0, h0 + hn)

                    def cpm(m):
                        i = MS.index(m)
                        return cp[:, i : i + 1, hs].to_broadcast([128, 2, hn, W])

                    av = acc[:, :, hs]
                    nc.vector.tensor_mul(av, ctr(P0)[:, :, hs], cpm(0))
                    # m=8 mid-chain (Pool's S8 lands early), m=5 last (Pool
                    # has ~a slab of slack)
                    tv = None
                    for Sx, m in ((S1, 1), (S2, 2), (S8, 8), (S4, 4), (S5, 5)):
                        tv = tmp.tile([128, 2, nr, W], dt.bfloat16, name="t", bufs=2)
                        nc.vector.tensor_mul(tv[:, :, hs], Sx[:, :, hs], cpm(m))
                        if m != 5:
                            nc.vector.tensor_add(av, av, tv[:, :, hs])
                    for a, b in slices:
                        nc.vector.tensor_add(
                            sm[:, :, 2 + r0 + a : 2 + r0 + b, 2 : W + 2],
                            acc[:, :, a:b],
                            tv[:, :, a:b],
                        )
                        if flush_fn is not None:
                            flush_fn(r0 + b)

            def rhs_ap(ki, q, rr, gn):
                dh, dw = OFFS[q // 3], OFFS[q % 3]
                return sm[:, ki, 2 + rr + dh : 2 + rr + gn + dh, 2 + dw : 2 + dw + W]

            def evac(pc, oi, rr, gn):
                ob = outp.tile([128, gn, W], ydt, name=f"ob{gn}",
                               bufs=(4 if gn == 5 else 2))
                nc.scalar.activation(
                    ob, pc, mybir.ActivationFunctionType.Relu,
                    bias=b_sb[:, oi : oi + 1], scale=1.0,
                )
                nc.sync.dma_start(out=y[oi, :, rr : rr + gn, :], in_=ob)

            def conv_group(groups):
                # groups: (rr, gn) output-row groups whose sm rows are ready
                if worder:
                    # weights-outer: one lhsT serves len(groups) consecutive
                    # matmuls (walrus-level weight reuse), psum banks rotate
                    for oi in range(2):
                        pcs = [
                            psp.tile([128, gn, W], dt.float32, name=f"pc{gn}", bufs=((4 if s5 == "pe" else 6) if gn == 5 else 1))
                            for rr, gn in groups
                        ]
                        for idx in range(18):
                            ki, q = idx // 9, idx % 9
                            lhsT = w_sb[:, ki, (q * 2 + oi) * 128 : (q * 2 + oi + 1) * 128]
                            for j, (rr, gn) in enumerate(groups):
                                nc.tensor.matmul(
                                    pcs[j], lhsT, rhs_ap(ki, q, rr, gn),
                                    start=(idx == 0), stop=(idx == 17),
                                )
                        for j, (rr, gn) in enumerate(groups):
                            evac(pcs[j], oi, rr, gn)
                else:
                    for oi in range(2):
                        for rr, gn in groups:
                            nb = (4 if s5 == "pe" else 6) if gn == 5 else 1
                            pc = psp.tile([128, gn, W], dt.float32, name=f"pc{gn}", bufs=nb)
                            for idx in range(18):
                                ki, q = idx // 9, idx % 9
                                lhsT = w_sb[:, ki, (q * 2 + oi) * 128 : (q * 2 + oi + 1) * 128]
                                nc.tensor.matmul(
                                    pc, lhsT, rhs_ap(ki, q, rr, gn),
                                    start=(idx == 0), stop=(idx == 17),
                                )
                            evac(pc, oi, rr, gn)

            def body():
                pending = list(cgroups)

                def flush(upto):
                    # group (rr, gn) reads sm rows rr-2 .. rr+gn+1 (dilated
                    # taps); rows 0..upto-1 have been written
                    ready = [g for g in pending if min(g[0] + g[1] + 2, H) <= upto]
                    for g in ready:
                        pending.remove(g)
                    if ready:
                        conv_group(ready)

                load_consts()
                for r0, nr, parts in slab_list:
                    smooth(r0, nr, flush_fn=flush, parts=parts)
                assert not pending

            if loop is not None:
                # `repeats` bodies unrolled inside the HW loop: consecutive
                # bodies overlap through the Tile dataflow (fill/tail hiding),
                # the For_i back-edge only serializes once per `repeats`.
                with tc.For_i(0, loop, 1):
                    for _ in range(repeats):
                        body()
            else:
                for _ in range(repeats):
                    body()

    nc.compile()
    return nc


def _prep(inputs):
    x = np.asarray(inputs["x"], np.float32)
    pm = np.asarray(inputs["perspective_map"], np.float32)
    co = np.asarray(inputs["sigma_coeffs"], np.float32)
    Wc = np.asarray(inputs["conv_w"], np.float32)
    bb = np.asarray(inputs["conv_b"], np.float32)

    # per-pixel coefficient planes (host): c_m = t^m / Z, replicated over partitions
    p = pm[:, 0]  # [B,H,W]
    sigma = co[0] * p**3 + co[1] * p**2 + co[2] * p + co[3]
    sigma = np.maximum(sigma, 0.5)
    t = np.exp(-1.0 / (2.0 * sigma * sigma))
    Z = 1 + 4 * t + 4 * t**2 + 4 * t**4 + 8 * t**5 + 4 * t**8
    cm = np.stack([(t**m) / Z for m in MS], axis=1).astype(BF16)  # [B,6,H,W]
    cpl = np.ascontiguousarray(np.broadcast_to(cm[:, None], (B, 128, 6, H, W)))

    # zero-padded bf16 input: [B, 128(part), 2(ct), HP, WP]
    xpad = np.zeros((B, 128, 2, HP, WP), BF16)
    xpad[:, :, :, 2 : H + 2, 2 : W + 2] = (
        x.astype(BF16).reshape(B, 2, 128, H, W).transpose(0, 2, 1, 3, 4)
    )

    # conv weights: lhsT layout [ki, 128(i), q, oi, 128(o)]
    Wt = Wc.transpose(1, 0, 2, 3).astype(BF16)  # [I, O, kh, kw]
    wts = np.empty((2, 128, 9, 2, 128), BF16)
    for ki in range(2):
        for q in range(9):
            kh, kw = q // 3, q % 3
            for oi in range(2):
                wts[ki, :, q, oi, :] = Wt[
                    ki * 128 : (ki + 1) * 128, oi * 128 : (oi + 1) * 128, kh, kw
                ]
    wts = wts.reshape(2, 128, 9 * 2 * 128)
    bias_h = np.ascontiguousarray(bb.reshape(2, 128).T.astype(np.float32))  # [128, 2]
    ident = np.eye(128, dtype=BF16)

    return [
        {"xp": xpad[b], "cpl": cpl[b], "wts": wts, "bias": bias_h, "ident": ident}
        for b in range(B)
    ]


def _get_nc(repeats=1, loop=None, s5="pe", worder=True, chunk=5, wq="act", yf32=False, slabs="s7", slices="fine"):
    key = ("nc", repeats, loop, s5, worder, chunk, wq, yf32, slabs, slices)
    if key not in _cache:
        _cache[key] = _build(repeats, loop, s5, worder, chunk, wq, yf32, slabs, slices)
    return _cache[key]


def run(inputs, trace=False, **kw):
    from concourse.bass_utils import run_bass_kernel_spmd

    nc = _get_nc()
    in_maps = _prep(inputs)
    res = run_bass_kernel_spmd(nc, in_maps, core_ids=list(range(B)), trace=trace, **kw)
    out = np.stack([r["y"].reshape(C, H, W) for r in res.results]).astype(np.float32)
    return out, res


def kernel(**inputs):
    out, _ = run(inputs)
    return out


# revision 35
# speedup vs baseline: 425.2469x; 1.0046x over previous
"""Trainium2 Bass kernel for BasicPGCBlock:
   per-pixel Gaussian smoothing (5x5, sigma = cubic(perspective)) -> dilated 3x3 conv (256->256) + bias + ReLU.

Sharding: data-parallel over batch, 1 image per NeuronCore (8 cores).

Math: the per-pixel 5x5 kernel w(u,v) = exp(-(u^2+v^2)/(2 s^2)) / Z factors through
t = exp(-1/(2 s^2)):  w(u,v) = t^(u^2+v^2) / Z, and u^2+v^2 in {0,1,2,4,5,8}.
So smoothed = sum_m c_m * S_m with c_m = t^m / Z (host-computed per-pixel planes,
replicated across partitions) and S_m = fixed 0/1 stencil sums of x built from
shifted adds (separable structure).

Engine split (DVE and PE are the co-bottlenecks, ~190us busy each):
 - PE: conv as 5-row output groups (N=480 moving, 720 matmuls vs 864 at 4-row)
   plus the 4-plane S5 stencil via identity-matmul PSUM accumulation (offloads
   DVE, the busiest engine; measured faster than gpsimd/DVE alternatives).
 - DVE: P1/P2 column-pair sums + S1/S2/S4/S8 stencils + the 11-op c_m MAC
   chain, all bf16 (2x DVE mode, ~0.52 ns/elem).
 - Act: PSUM evacuation with fused bias+ReLU to bf16 (halves y DMA), and the
   conv-weight DMA on the Act queue so it never queues behind the input stream.
 - gpsimd/Pool: unused for compute — measured ~4x slower than the cost model
   on HW and it serialized the pipeline.

Scheduling: 8/16/.../16/8-row slabs; the final MAC add of each slab is emitted
in ~5-row slices with the conv flush between slices, releasing conv groups
every ~5 smoothed rows (smooth PE feed, short fill). The last slab applies the
MAC chain in 6+2-row parts so the only conv work gated on the final smoothed
rows is one 4-row group (~9us tail). Measured HW notes: unrolling multiple
bodies inside the For_i timing loop is ~25% SLOWER on HW (instruction-stream
pressure the cost model does not see), fp8 DoubleRow matmul would halve PE
time but fails the 2e-2 accuracy gate, and DVE fast mode is already engaged
(bf16, packed, SBUF).
"""

import sys

sys.path.insert(0, "/opt/trn_rl_repo")

import numpy as np
import ml_dtypes

BF16 = ml_dtypes.bfloat16

B, C, H, W = 8, 256, 96, 96
HP, WP = H + 4, W + 4          # zero-padded by 2 on each side
OFFS = (-2, 0, 2)              # dilated conv offsets
MS = (0, 1, 2, 4, 5, 8)        # exponents of t present in the 5x5 kernel
# conv output row-groups (start, nrows): 18x5 + (90,2) + (92,4); the final
# 4-row group is the only conv work gated on the last 2 smoothed rows.
CGROUPS = tuple((i * 5, 5) for i in range(18)) + ((90, 2), (92, 4))

_cache = {}


def _build(repeats=1, loop=None, s5="pe", worder=True, chunk=5, wq="act", yf32=False, slabs="s7", slices="fine", oq="sync"):
    import concourse.mybir as mybir
    from concourse import bacc
    from concourse.tile import TileContext

    if chunk == 5:
        cgroups = CGROUPS
    else:
        cgroups = tuple((i * 4, 4) for i in range(24))

    def mid_parts(nr):
        if slices == "fine":
            bnds = ((0, 6), (6, 11), (11, 16)) if nr == 16 else ((0, 8),)
            return ((0, nr, bnds),)
        return ((0, nr, ((0, nr),)),)

    if slabs == "s7":
        slab_list = [(0, 8, mid_parts(8))] + [
            (r, 16, mid_parts(16)) for r in (8, 24, 40, 56, 72)
        ] + [(88, 8, ((0, 6, (((0, 4), (4, 6)) if slices == "fine" else ((0, 6),))),
                      (6, 2, ((6, 8),))))]
    else:
        slab_list = [(r, 16, mid_parts(16)) for r in (0, 16, 32, 48, 64)] + [
            (80, 16, ((0, 14, (((0, 6), (6, 11), (11, 14)) if slices == "fine" else ((0, 14),))),
                      (14, 2, ((14, 16),))))
        ]
    dt = mybir.dt
    nc = bacc.Bacc("TRN2", target_bir_lowering=False, debug=False)

    xp = nc.dram_tensor("xp", (128, 2, HP, WP), dt.bfloat16, kind="ExternalInput").ap()
    cpl = nc.dram_tensor("cpl", (128, 6, H, W), dt.bfloat16, kind="ExternalInput").ap()
    wts = nc.dram_tensor("wts", (2, 128, 9 * 2 * 128), dt.bfloat16, kind="ExternalInput").ap()
    bias = nc.dram_tensor("bias", (128, 2), dt.float32, kind="ExternalInput").ap()
    ident = nc.dram_tensor("ident", (128, 128), dt.bfloat16, kind="ExternalInput").ap()
    ydt = dt.float32 if yf32 else dt.bfloat16
    y = nc.dram_tensor("y", (2, 128, H, W), ydt, kind="ExternalOutput").ap()

    with TileContext(nc) as tc:
        with (
            tc.tile_pool(name="const", bufs=1) as constp,
            tc.tile_pool(name="smpool", bufs=1) as smpool,
            tc.tile_pool(name="io", bufs=2) as iop,
            tc.tile_pool(name="tmp", bufs=1) as tmp,
            tc.tile_pool(name="outp", bufs=1) as outp,
            tc.tile_pool(name="psum", bufs=1, space="PSUM") as psp,
        ):
            w_sb = constp.tile([128, 2, 9 * 2 * 128], dt.bfloat16)
            b_sb = constp.tile([128, 2], dt.float32)
            id_sb = constp.tile([128, 128], dt.bfloat16)
            if s5 == "pe":
                nc.sync.dma_start(out=id_sb, in_=ident)

            def load_consts():
                # Activation-engine DMA queue: runs in parallel with the SP
                # queue that carries the (much larger) xs/cp input stream, so
                # the first conv group is never gated on the weights landing.
                dq = nc.scalar if wq == "act" else nc.sync
                dq.dma_start(out=w_sb[:, 0], in_=wts[0])
                dq.dma_start(out=w_sb[:, 1], in_=wts[1])
                dq.dma_start(out=b_sb, in_=bias)

            sm = smpool.tile([128, 2, HP, WP], dt.bfloat16)
            # zero only the 2-wide pad ring; the interior is fully rewritten
            nc.vector.memset(sm[:, :, 0:2, :], 0.0)
            nc.vector.memset(sm[:, :, HP - 2 : HP, :], 0.0)
            nc.vector.memset(sm[:, :, 2 : HP - 2, 0:2], 0.0)
            nc.vector.memset(sm[:, :, 2 : HP - 2, WP - 2 : WP], 0.0)

            def smooth(r0, nr, flush_fn=None, parts=None):
                xs = iop.tile([128, 2, nr + 4, WP], dt.bfloat16, name="xs")
                nc.sync.dma_start(out=xs, in_=xp[:, :, r0 : r0 + nr + 4, :])
                cp = iop.tile([128, 6, nr, W], dt.bfloat16, name="cp")
                nc.sync.dma_start(out=cp, in_=cpl[:, :, r0 : r0 + nr, :])

                P0 = xs[:, :, :, 2 : W + 2]
                P1 = tmp.tile([128, 2, nr + 4, W], dt.bfloat16, name="P1", bufs=2)
                nc.vector.tensor_add(P1, xs[:, :, :, 1 : W + 1], xs[:, :, :, 3 : W + 3])
                P2 = tmp.tile([128, 2, nr + 4, W], dt.bfloat16, name="P2", bufs=2)
                nc.vector.tensor_add(P2, xs[:, :, :, 0:W], xs[:, :, :, 4 : W + 4])

                ctr = lambda P: P[:, :, 2 : nr + 2]
                u1 = lambda P: P[:, :, 1 : nr + 1]
                d1 = lambda P: P[:, :, 3 : nr + 3]
                u2 = lambda P: P[:, :, 0:nr]
                d2 = lambda P: P[:, :, 4 : nr + 4]

                # S5 = (P1[h-2]+P1[h+2]) + (P2[h-1]+P2[h+1]): on PE via
                # identity-matmul PSUM accumulation (offloads the busiest
                # engine, DVE), with gpsimd/DVE fallbacks for A/B testing.
                S5 = tmp.tile([128, 2, nr, W], dt.bfloat16, name="S5", bufs=2)
                S8 = tmp.tile([128, 2, nr, W], dt.bfloat16, name="S8", bufs=2)
                if s5 == "pe":
                    nc.vector.tensor_add(S8, u2(P2), d2(P2))
                    for ct in range(2):
                        for rs in range(0, nr, 4):
                            pc5 = psp.tile([128, 4, W], dt.float32, name="ps5", bufs=2)
                            for j, Pv in enumerate((u2(P1), d2(P1), u1(P2), d1(P2))):
                                nc.tensor.matmul(
                                    pc5, id_sb, Pv[:, ct, rs : rs + 4, :],
                                    start=(j == 0), stop=(j == 3),
                                )
                            nc.scalar.activation(
                                S5[:, ct, rs : rs + 4, :], pc5,
                                mybir.ActivationFunctionType.Copy,
                            )
                else:
                    eng = nc.gpsimd if s5 == "pool" else nc.vector
                    eng.tensor_add(S8, u2(P2), d2(P2))
                    Qp = tmp.tile([128, 2, nr, W], dt.bfloat16, name="Qp", bufs=2)
                    eng.tensor_add(S5, u2(P1), d2(P1))
                    eng.tensor_add(Qp, u1(P2), d1(P2))
                    eng.tensor_add(S5, S5, Qp)

                S1 = tmp.tile([128, 2, nr, W], dt.bfloat16, name="S1")
                nc.vector.tensor_add(S1, u1(P0), d1(P0))
                nc.vector.tensor_add(S1, S1, ctr(P1))
                S2 = tmp.tile([128, 2, nr, W], dt.bfloat16, name="S2")
                nc.vector.tensor_add(S2, u1(P1), d1(P1))
                S4 = tmp.tile([128, 2, nr, W], dt.bfloat16, name="S4")
                nc.vector.tensor_add(S4, u2(P0), d2(P0))
                nc.vector.tensor_add(S4, S4, ctr(P2))

                acc = tmp.tile([128, 2, nr, W], dt.bfloat16, name="acc")

                for h0, hn, slices in parts:
                    hs = slice(h0, h0 + hn)

                    def cpm(m):
                        i = MS.index(m)
                        return cp[:, i : i + 1, hs].to_broadcast([128, 2, hn, W])

                    av = acc[:, :, hs]
                    nc.vector.tensor_mul(av, ctr(P0)[:, :, hs], cpm(0))
                    # m=8 mid-chain (Pool's S8 lands early), m=5 last (Pool
                    # has ~a slab of slack)
                    tv = None
                    for Sx, m in ((S1, 1), (S2, 2), (S8, 8), (S4, 4), (S5, 5)):
                        tv = tmp.tile([128, 2, nr, W], dt.bfloat16, name="t", bufs=2)
                        nc.vector.tensor_mul(tv[:, :, hs], Sx[:, :, hs], cpm(m))
                        if m != 5:
                            nc.vector.tensor_add(av, av, tv[:, :, hs])
                    for a, b in slices:
                        nc.vector.tensor_add(
                            sm[:, :, 2 + r0 + a : 2 + r0 + b, 2 : W + 2],
                            acc[:, :, a:b],
                            tv[:, :, a:b],
                        )
                        if flush_fn is not None:
                            flush_fn(r0 + b)

            def rhs_ap(ki, q, rr, gn):
                dh, dw = OFFS[q // 3], OFFS[q % 3]
                return sm[:, ki, 2 + rr + dh : 2 + rr + gn + dh, 2 + dw : 2 + dw + W]

            def evac(pc, oi, rr, gn):
                ob = outp.tile([128, gn, W], ydt, name=f"ob{gn}",
                               bufs=(4 if gn == 5 else 2))
                nc.scalar.activation(
                    ob, pc, mybir.ActivationFunctionType.Relu,
                    bias=b_sb[:, oi : oi + 1], scale=1.0,
                )
                odq = nc.gpsimd if oq == "pool" else nc.sync
                odq.dma_start(out=y[oi, :, rr : rr + gn, :], in_=ob)

            def conv_group(groups):
                # groups: (rr, gn) output-row groups whose sm rows are ready
                if worder:
                    # weights-outer: one lhsT serves len(groups) consecutive
                    # matmuls (walrus-level weight reuse), psum banks rotate
                    for oi in range(2):
                        pcs = [
                            psp.tile([128, gn, W], dt.float32, name=f"pc{gn}", bufs=((4 if s5 == "pe" else 6) if gn == 5 else 1))
                            for rr, gn in groups
                        ]
                        for idx in range(18):
                            ki, q = idx // 9, idx % 9
                            lhsT = w_sb[:, ki, (q * 2 + oi) * 128 : (q * 2 + oi + 1) * 128]
                            for j, (rr, gn) in enumerate(groups):
                                nc.tensor.matmul(
                                    pcs[j], lhsT, rhs_ap(ki, q, rr, gn),
                                    start=(idx == 0), stop=(idx == 17),
                                )
                        for j, (rr, gn) in enumerate(groups):
                            evac(pcs[j], oi, rr, gn)
                else:
                    for oi in range(2):
                        for rr, gn in groups:
                            nb = (4 if s5 == "pe" else 6) if gn == 5 else 1
                            pc = psp.tile([128, gn, W], dt.float32, name=f"pc{gn}", bufs=nb)
                            for idx in range(18):
                                ki, q = idx // 9, idx % 9
                                lhsT = w_sb[:, ki, (q * 2 + oi) * 128 : (q * 2 + oi + 1) * 128]
                                nc.tensor.matmul(
                                    pc, lhsT, rhs_ap(ki, q, rr, gn),
                                    start=(idx == 0), stop=(idx == 17),
                                )
                            evac(pc, oi, rr, gn)

            def body():
                pending = list(cgroups)

                def flush(upto):
                    # group (rr, gn) reads sm rows rr-2 .. rr+gn+1 (dilated
                    # taps); rows 0..upto-1 have been written
                    ready = [g for g in pending if min(g[0] + g[1] + 2, H) <= upto]
                    for g in ready:
                        pending.remove(g)
                    if ready:
                        conv_group(ready)

                load_consts()
                for r0, nr, parts in slab_list:
                    smooth(r0, nr, flush_fn=flush, parts=parts)
                assert not pending

            if loop is not None:
                # `repeats` bodies unrolled inside the HW loop: consecutive
                # bodies overlap through the Tile dataflow (fill/tail hiding),
                # the For_i back-edge only serializes once per `repeats`.
                with tc.For_i(0, loop, 1):
                    for _ in range(repeats):
                        body()
            else:
                for _ in range(repeats):
                    body()

    nc.compile()
    return nc


def _prep(inputs):
    x = np.asarray(inputs["x"], np.float32)
    pm = np.asarray(inputs["perspective_map"], np.float32)
    co = np.asarray(inputs["sigma_coeffs"], np.float32)
    Wc = np.asarray(inputs["conv_w"], np.float32)
    bb = np.asarray(inputs["conv_b"], np.float32)

    # per-pixel coefficient planes (host): c_m = t^m / Z, replicated over partitions
    p = pm[:, 0]  # [B,H,W]
    sigma = co[0] * p**3 + co[1] * p**2 + co[2] * p + co[3]
    sigma = np.maximum(sigma, 0.5)
    t = np.exp(-1.0 / (2.0 * sigma * sigma))
    Z = 1 + 4 * t + 4 * t**2 + 4 * t**4 + 8 * t**5 + 4 * t**8
    cm = np.stack([(t**m) / Z for m in MS], axis=1).astype(BF16)  # [B,6,H,W]
    cpl = np.ascontiguousarray(np.broadcast_to(cm[:, None], (B, 128, 6, H, W)))

    # zero-padded bf16 input: [B, 128(part), 2(ct), HP, WP]
    xpad = np.zeros((B, 128, 2, HP, WP), BF16)
    xpad[:, :, :, 2 : H + 2, 2 : W + 2] = (
        x.astype(BF16).reshape(B, 2, 128, H, W).transpose(0, 2, 1, 3, 4)
    )

    # conv weights: lhsT layout [ki, 128(i), q, oi, 128(o)]
    Wt = Wc.transpose(1, 0, 2, 3).astype(BF16)  # [I, O, kh, kw]
    wts = np.empty((2, 128, 9, 2, 128), BF16)
    for ki in range(2):
        for q in range(9):
            kh, kw = q // 3, q % 3
            for oi in range(2):
                wts[ki, :, q, oi, :] = Wt[
                    ki * 128 : (ki + 1) * 128, oi * 128 : (oi + 1) * 128, kh, kw
                ]
    wts = wts.reshape(2, 128, 9 * 2 * 128)
    bias_h = np.ascontiguousarray(bb.reshape(2, 128).T.astype(np.float32))  # [128, 2]
    ident = np.eye(128, dtype=BF16)

    return [
        {"xp": xpad[b], "cpl": cpl[b], "wts": wts, "bias": bias_h, "ident": ident}
        for b in range(B)
    ]


def _get_nc(repeats=1, loop=None, s5="pe", worder=True, chunk=5, wq="act", yf32=False, slabs="s7", slices="fine", oq="sync"):
    key = ("nc", repeats, loop, s5, worder, chunk, wq, yf32, slabs, slices, oq)
    if key not in _cache:
        _cache[key] = _build(repeats, loop, s5, worder, chunk, wq, yf32, slabs, slices, oq)
    return _cache[key]


def run(inputs, trace=False, **kw):
    from concourse.bass_utils import run_bass_kernel_spmd

    nc = _get_nc()
    in_maps = _prep(inputs)
    res = run_bass_kernel_spmd(nc, in_maps, core_ids=list(range(B)), trace=trace, **kw)
    out = np.stack([r["y"].reshape(C, H, W) for r in res.results]).astype(np.float32)
    return out, res


def kernel(**inputs):
    out, _ = run(inputs)
    return out


# revision 37
# speedup vs baseline: 428.7146x; 1.0082x over previous
"""Trainium2 Bass kernel for BasicPGCBlock:
   per-pixel Gaussian smoothing (5x5, sigma = cubic(perspective)) -> dilated 3x3 conv (256->256) + bias + ReLU.

Sharding: data-parallel over batch, 1 image per NeuronCore (8 cores).

Math: the per-pixel 5x5 kernel w(u,v) = exp(-(u^2+v^2)/(2 s^2)) / Z factors through
t = exp(-1/(2 s^2)):  w(u,v) = t^(u^2+v^2) / Z, and u^2+v^2 in {0,1,2,4,5,8}.
So smoothed = sum_m c_m * S_m with c_m = t^m / Z (host-computed per-pixel planes,
replicated across partitions) and S_m = fixed 0/1 stencil sums of x built from
shifted adds (separable structure).

Engine split (DVE and PE are the co-bottlenecks, ~190us busy each):
 - PE: conv as 5-row output groups (N=480 moving, 720 matmuls vs 864 at 4-row)
   plus the 4-plane S5 stencil via identity-matmul PSUM accumulation (offloads
   DVE, the busiest engine; measured faster than gpsimd/DVE alternatives).
 - DVE: P1/P2 column-pair sums + S1/S2/S4/S8 stencils + the 11-op c_m MAC
   chain, all bf16 (2x DVE mode, ~0.52 ns/elem).
 - Act: PSUM evacuation with fused bias+ReLU to bf16 (halves y DMA), and the
   conv-weight DMA on the Act queue so it never queues behind the input stream.
 - gpsimd/Pool: unused for compute — measured ~4x slower than the cost model
   on HW and it serialized the pipeline.

Scheduling: 8/16/.../16/8-row slabs; the final MAC add of each slab is emitted
in ~5-row slices with the conv flush between slices, releasing conv groups
every ~5 smoothed rows (smooth PE feed, short fill). The last slab applies the
MAC chain in 6+2-row parts so the only conv work gated on the final smoothed
rows is one 4-row group (~9us tail). Measured HW notes: unrolling multiple
bodies inside the For_i timing loop is ~25% SLOWER on HW (instruction-stream
pressure the cost model does not see), fp8 DoubleRow matmul would halve PE
time but fails the 2e-2 accuracy gate, and DVE fast mode is already engaged
(bf16, packed, SBUF).
"""

import sys

sys.path.insert(0, "/opt/trn_rl_repo")

import numpy as np
import ml_dtypes

BF16 = ml_dtypes.bfloat16

B, C, H, W = 8, 256, 96, 96
HP, WP = H + 4, W + 4          # zero-padded by 2 on each side
OFFS = (-2, 0, 2)              # dilated conv offsets
MS = (0, 1, 2, 4, 5, 8)        # exponents of t present in the 5x5 kernel
# conv output row-groups (start, nrows): 18x5 + (90,2) + (92,4); the final
# 4-row group is the only conv work gated on the last 2 smoothed rows.
CGROUPS = tuple((i * 5, 5) for i in range(18)) + ((90, 2), (92, 4))

_cache = {}


def _build(repeats=1, loop=None, s5="pe", worder=True, chunk=5, wq="act", yf32=False, slabs="s7", slices="fine", oq="sync", s8pe=False, s2pe=False):
    import concourse.mybir as mybir
    from concourse import bacc
    from concourse.tile import TileContext

    if chunk == 5:
        cgroups = CGROUPS
    else:
        cgroups = tuple((i * 4, 4) for i in range(24))

    def mid_parts(nr):
        if slices == "fine":
            bnds = ((0, 6), (6, 11), (11, 16)) if nr == 16 else ((0, 8),)
            return ((0, nr, bnds),)
        return ((0, nr, ((0, nr),)),)

    if slabs == "s7":
        slab_list = [(0, 8, mid_parts(8))] + [
            (r, 16, mid_parts(16)) for r in (8, 24, 40, 56, 72)
        ] + [(88, 8, ((0, 6, (((0, 4), (4, 6)) if slices == "fine" else ((0, 6),))),
                      (6, 2, ((6, 8),))))]
    else:
        slab_list = [(r, 16, mid_parts(16)) for r in (0, 16, 32, 48, 64)] + [
            (80, 16, ((0, 14, (((0, 6), (6, 11), (11, 14)) if slices == "fine" else ((0, 14),))),
                      (14, 2, ((14, 16),))))
        ]
    dt = mybir.dt
    nc = bacc.Bacc("TRN2", target_bir_lowering=False, debug=False)

    xp = nc.dram_tensor("xp", (128, 2, HP, WP), dt.bfloat16, kind="ExternalInput").ap()
    cpl = nc.dram_tensor("cpl", (128, 6, H, W), dt.bfloat16, kind="ExternalInput").ap()
    wts = nc.dram_tensor("wts", (2, 128, 9 * 2 * 128), dt.bfloat16, kind="ExternalInput").ap()
    bias = nc.dram_tensor("bias", (128, 2), dt.float32, kind="ExternalInput").ap()
    ident = nc.dram_tensor("ident", (128, 128), dt.bfloat16, kind="ExternalInput").ap()
    ydt = dt.float32 if yf32 else dt.bfloat16
    y = nc.dram_tensor("y", (2, 128, H, W), ydt, kind="ExternalOutput").ap()

    with TileContext(nc) as tc:
        with (
            tc.tile_pool(name="const", bufs=1) as constp,
            tc.tile_pool(name="smpool", bufs=1) as smpool,
            tc.tile_pool(name="io", bufs=2) as iop,
            tc.tile_pool(name="tmp", bufs=1) as tmp,
            tc.tile_pool(name="outp", bufs=1) as outp,
            tc.tile_pool(name="psum", bufs=1, space="PSUM") as psp,
        ):
            w_sb = constp.tile([128, 2, 9 * 2 * 128], dt.bfloat16)
            b_sb = constp.tile([128, 2], dt.float32)
            id_sb = constp.tile([128, 128], dt.bfloat16)
            if s5 == "pe":
                nc.sync.dma_start(out=id_sb, in_=ident)

            def load_consts():
                # Activation-engine DMA queue: runs in parallel with the SP
                # queue that carries the (much larger) xs/cp input stream, so
                # the first conv group is never gated on the weights landing.
                dq = nc.scalar if wq == "act" else nc.sync
                dq.dma_start(out=w_sb[:, 0], in_=wts[0])
                dq.dma_start(out=w_sb[:, 1], in_=wts[1])
                dq.dma_start(out=b_sb, in_=bias)

            sm = smpool.tile([128, 2, HP, WP], dt.bfloat16)
            # zero only the 2-wide pad ring; the interior is fully rewritten
            nc.vector.memset(sm[:, :, 0:2, :], 0.0)
            nc.vector.memset(sm[:, :, HP - 2 : HP, :], 0.0)
            nc.vector.memset(sm[:, :, 2 : HP - 2, 0:2], 0.0)
            nc.vector.memset(sm[:, :, 2 : HP - 2, WP - 2 : WP], 0.0)

            def smooth(r0, nr, flush_fn=None, parts=None):
                xs = iop.tile([128, 2, nr + 4, WP], dt.bfloat16, name="xs")
                nc.sync.dma_start(out=xs, in_=xp[:, :, r0 : r0 + nr + 4, :])
                cp = iop.tile([128, 6, nr, W], dt.bfloat16, name="cp")
                nc.sync.dma_start(out=cp, in_=cpl[:, :, r0 : r0 + nr, :])

                P0 = xs[:, :, :, 2 : W + 2]
                P1 = tmp.tile([128, 2, nr + 4, W], dt.bfloat16, name="P1", bufs=2)
                nc.vector.tensor_add(P1, xs[:, :, :, 1 : W + 1], xs[:, :, :, 3 : W + 3])
                P2 = tmp.tile([128, 2, nr + 4, W], dt.bfloat16, name="P2", bufs=2)
                nc.vector.tensor_add(P2, xs[:, :, :, 0:W], xs[:, :, :, 4 : W + 4])

                ctr = lambda P: P[:, :, 2 : nr + 2]
                u1 = lambda P: P[:, :, 1 : nr + 1]
                d1 = lambda P: P[:, :, 3 : nr + 3]
                u2 = lambda P: P[:, :, 0:nr]
                d2 = lambda P: P[:, :, 4 : nr + 4]

                # S5 = (P1[h-2]+P1[h+2]) + (P2[h-1]+P2[h+1]): on PE via
                # identity-matmul PSUM accumulation (offloads the busiest
                # engine, DVE), with gpsimd/DVE fallbacks for A/B testing.
                S5 = tmp.tile([128, 2, nr, W], dt.bfloat16, name="S5", bufs=2)
                S8 = tmp.tile([128, 2, nr, W], dt.bfloat16, name="S8", bufs=2)
                if s5 == "pe":
                    if s8pe:
                        for ct in range(2):
                            for rs in range(0, nr, 4):
                                pc8 = psp.tile([128, 4, W], dt.float32, name="ps5", bufs=2)
                                nc.tensor.matmul(pc8, id_sb, u2(P2)[:, ct, rs : rs + 4, :],
                                                 start=True, stop=False)
                                nc.tensor.matmul(pc8, id_sb, d2(P2)[:, ct, rs : rs + 4, :],
                                                 start=False, stop=True)
                                nc.scalar.activation(
                                    S8[:, ct, rs : rs + 4, :], pc8,
                                    mybir.ActivationFunctionType.Copy,
                                )
                    else:
                        nc.vector.tensor_add(S8, u2(P2), d2(P2))
                    for ct in range(2):
                        for rs in range(0, nr, 4):
                            pc5 = psp.tile([128, 4, W], dt.float32, name="ps5", bufs=2)
                            for j, Pv in enumerate((u2(P1), d2(P1), u1(P2), d1(P2))):
                                nc.tensor.matmul(
                                    pc5, id_sb, Pv[:, ct, rs : rs + 4, :],
                                    start=(j == 0), stop=(j == 3),
                                )
                            nc.scalar.activation(
                                S5[:, ct, rs : rs + 4, :], pc5,
                                mybir.ActivationFunctionType.Copy,
                            )
                else:
                    eng = nc.gpsimd if s5 == "pool" else nc.vector
                    eng.tensor_add(S8, u2(P2), d2(P2))
                    Qp = tmp.tile([128, 2, nr, W], dt.bfloat16, name="Qp", bufs=2)
                    eng.tensor_add(S5, u2(P1), d2(P1))
                    eng.tensor_add(Qp, u1(P2), d1(P2))
                    eng.tensor_add(S5, S5, Qp)

                S1 = tmp.tile([128, 2, nr, W], dt.bfloat16, name="S1")
                nc.vector.tensor_add(S1, u1(P0), d1(P0))
                nc.vector.tensor_add(S1, S1, ctr(P1))
                S2 = tmp.tile([128, 2, nr, W], dt.bfloat16, name="S2", bufs=2)
                if s2pe:
                    for ct in range(2):
                        for rs in range(0, nr, 4):
                            pc2s = psp.tile([128, 4, W], dt.float32, name="ps5", bufs=2)
                            nc.tensor.matmul(pc2s, id_sb, u1(P1)[:, ct, rs : rs + 4, :],
                                             start=True, stop=False)
                            nc.tensor.matmul(pc2s, id_sb, d1(P1)[:, ct, rs : rs + 4, :],
                                             start=False, stop=True)
                            nc.scalar.activation(
                                S2[:, ct, rs : rs + 4, :], pc2s,
                                mybir.ActivationFunctionType.Copy,
                            )
                else:
                    nc.vector.tensor_add(S2, u1(P1), d1(P1))
                S4 = tmp.tile([128, 2, nr, W], dt.bfloat16, name="S4")
                nc.vector.tensor_add(S4, u2(P0), d2(P0))
                nc.vector.tensor_add(S4, S4, ctr(P2))

                acc = tmp.tile([128, 2, nr, W], dt.bfloat16, name="acc")

                for h0, hn, slices in parts:
                    hs = slice(h0, h0 + hn)

                    def cpm(m):
                        i = MS.index(m)
                        return cp[:, i : i + 1, hs].to_broadcast([128, 2, hn, W])

                    av = acc[:, :, hs]
                    nc.vector.tensor_mul(av, ctr(P0)[:, :, hs], cpm(0))
                    # m=8 mid-chain (Pool's S8 lands early), m=5 last (Pool
                    # has ~a slab of slack)
                    tv = None
                    for Sx, m in ((S1, 1), (S2, 2), (S8, 8), (S4, 4), (S5, 5)):
                        tv = tmp.tile([128, 2, nr, W], dt.bfloat16, name="t", bufs=2)
                        nc.vector.tensor_mul(tv[:, :, hs], Sx[:, :, hs], cpm(m))
                        if m != 5:
                            nc.vector.tensor_add(av, av, tv[:, :, hs])
                    for a, b in slices:
                        nc.vector.tensor_add(
                            sm[:, :, 2 + r0 + a : 2 + r0 + b, 2 : W + 2],
                            acc[:, :, a:b],
                            tv[:, :, a:b],
                        )
                        if flush_fn is not None:
                            flush_fn(r0 + b)

            def rhs_ap(ki, q, rr, gn):
                dh, dw = OFFS[q // 3], OFFS[q % 3]
                return sm[:, ki, 2 + rr + dh : 2 + rr + gn + dh, 2 + dw : 2 + dw + W]

            def evac(pc, oi, rr, gn):
                ob = outp.tile([128, gn, W], ydt, name=f"ob{gn}",
                               bufs=(4 if gn == 5 else 2))
                nc.scalar.activation(
                    ob, pc, mybir.ActivationFunctionType.Relu,
                    bias=b_sb[:, oi : oi + 1], scale=1.0,
                )
                odq = nc.gpsimd if oq == "pool" else nc.sync
                odq.dma_start(out=y[oi, :, rr : rr + gn, :], in_=ob)

            def conv_group(groups):
                # groups: (rr, gn) output-row groups whose sm rows are ready
                if worder:
                    # weights-outer: one lhsT serves len(groups) consecutive
                    # matmuls (walrus-level weight reuse), psum banks rotate
                    for oi in range(2):
                        pcs = [
                            psp.tile([128, gn, W], dt.float32, name=f"pc{gn}", bufs=((4 if s5 == "pe" else 6) if gn == 5 else 1))
                            for rr, gn in groups
                        ]
                        for idx in range(18):
                            ki, q = idx // 9, idx % 9
                            lhsT = w_sb[:, ki, (q * 2 + oi) * 128 : (q * 2 + oi + 1) * 128]
                            for j, (rr, gn) in enumerate(groups):
                                nc.tensor.matmul(
                                    pcs[j], lhsT, rhs_ap(ki, q, rr, gn),
                                    start=(idx == 0), stop=(idx == 17),
                                )
                        for j, (rr, gn) in enumerate(groups):
                            evac(pcs[j], oi, rr, gn)
                else:
                    for oi in range(2):
                        for rr, gn in groups:
                            nb = (4 if s5 == "pe" else 6) if gn == 5 else 1
                            pc = psp.tile([128, gn, W], dt.float32, name=f"pc{gn}", bufs=nb)
                            for idx in range(18):
                                ki, q = idx // 9, idx % 9
                                lhsT = w_sb[:, ki, (q * 2 + oi) * 128 : (q * 2 + oi + 1) * 128]
                                nc.tensor.matmul(
                                    pc, lhsT, rhs_ap(ki, q, rr, gn),
                                    start=(idx == 0), stop=(idx == 17),
                                )
                            evac(pc, oi, rr, gn)

            def body():
                pending = list(cgroups)

                def flush(upto):
                    # group (rr, gn) reads sm rows rr-2 .. rr+gn+1 (dilated
                    # taps); rows 0..upto-1 have been written
                    ready = [g for g in pending if min(g[0] + g[1] + 2, H) <= upto]
                    for g in ready:
                        pending.remove(g)
                    if ready:
                        conv_group(ready)

                load_consts()
                for r0, nr, parts in slab_list:
                    smooth(r0, nr, flush_fn=flush, parts=parts)
                assert not pending

            if loop is not None:
                # `repeats` bodies unrolled inside the HW loop: consecutive
                # bodies overlap through the Tile dataflow (fill/tail hiding),
                # the For_i back-edge only serializes once per `repeats`.
                with tc.For_i(0, loop, 1):
                    for _ in range(repeats):
                        body()
            else:
                for _ in range(repeats):
                    body()

    nc.compile()
    return nc


def _prep(inputs):
    x = np.asarray(inputs["x"], np.float32)
    pm = np.asarray(inputs["perspective_map"], np.float32)
    co = np.asarray(inputs["sigma_coeffs"], np.float32)
    Wc = np.asarray(inputs["conv_w"], np.float32)
    bb = np.asarray(inputs["conv_b"], np.float32)

    # per-pixel coefficient planes (host): c_m = t^m / Z, replicated over partitions
    p = pm[:, 0]  # [B,H,W]
    sigma = co[0] * p**3 + co[1] * p**2 + co[2] * p + co[3]
    sigma = np.maximum(sigma, 0.5)
    t = np.exp(-1.0 / (2.0 * sigma * sigma))
    Z = 1 + 4 * t + 4 * t**2 + 4 * t**4 + 8 * t**5 + 4 * t**8
    cm = np.stack([(t**m) / Z for m in MS], axis=1).astype(BF16)  # [B,6,H,W]
    cpl = np.ascontiguousarray(np.broadcast_to(cm[:, None], (B, 128, 6, H, W)))

    # zero-padded bf16 input: [B, 128(part), 2(ct), HP, WP]
    xpad = np.zeros((B, 128, 2, HP, WP), BF16)
    xpad[:, :, :, 2 : H + 2, 2 : W + 2] = (
        x.astype(BF16).reshape(B, 2, 128, H, W).transpose(0, 2, 1, 3, 4)
    )

    # conv weights: lhsT layout [ki, 128(i), q, oi, 128(o)]
    Wt = Wc.transpose(1, 0, 2, 3).astype(BF16)  # [I, O, kh, kw]
    wts = np.empty((2, 128, 9, 2, 128), BF16)
    for ki in range(2):
        for q in range(9):
            kh, kw = q // 3, q % 3
            for oi in range(2):
                wts[ki, :, q, oi, :] = Wt[
                    ki * 128 : (ki + 1) * 128, oi * 128 : (oi + 1) * 128, kh, kw
                ]
    wts = wts.reshape(2, 128, 9 * 2 * 128)
    bias_h = np.ascontiguousarray(bb.reshape(2, 128).T.astype(np.float32))  # [128, 2]
    ident = np.eye(128, dtype=BF16)

    return [
        {"xp": xpad[b], "cpl": cpl[b], "wts": wts, "bias": bias_h, "ident": ident}
        for b in range(B)
    ]


def _get_nc(repeats=1, loop=None, s5="pe", worder=True, chunk=5, wq="act", yf32=False, slabs="s7", slices="fine", oq="sync", s8pe=False, s2pe=False):
    key = ("nc", repeats, loop, s5, worder, chunk, wq, yf32, slabs, slices, oq, s8pe, s2pe)
    if key not in _cache:
        _cache[key] = _build(repeats, loop, s5, worder, chunk, wq, yf32, slabs, slices, oq, s8pe, s2pe)
    return _cache[key]


def run(inputs, trace=False, **kw):
    from concourse.bass_utils import run_bass_kernel_spmd

    nc = _get_nc()
    in_maps = _prep(inputs)
    res = run_bass_kernel_spmd(nc, in_maps, core_ids=list(range(B)), trace=trace, **kw)
    out = np.stack([r["y"].reshape(C, H, W) for r in res.results]).astype(np.float32)
    return out, res


def kernel(**inputs):
    out, _ = run(inputs)
    return out
